# revision 1
# baseline (speedup 1.0000x reference)
"""CosArcLoss on 8 TRN2 NeuronCores (Bass/Tile).

Math (reference, f32):
    t_i   = preds[i, labels[i]]
    theta = arccos(clip(t_i, -1+1e-12, 1-1e-12))    # == clip(t_i,-1,1) in f32
    num_i = 30*(cos(theta + 0.5) - 0.35)
          = 30*cos(0.5)*t_i - 30*sin(0.5)*sqrt(1-t_i^2) - 10.5
    S_i   = sum_j exp(30*preds[i,j])
    den_i = exp(num_i) + S_i - exp(30*t_i)
    loss  = mean_i( log(den_i) - num_i )

Sharding: batch-parallel, 256 rows/core. Each row is rotated on the host so
its target column sits at local column 0 (row sums are rotation-invariant),
making the device program a pure streaming exp+rowsum with a tiny epilogue
and no gather / no collective. Final mean over the 8*[128,2] per-row losses
happens on the host (the "all-reduce" of the unshard step).

Schedule notes: the numerator chain (sqrt etc.) depends only on the target
column, so it is emitted first — its ACT table loads hide under the first
x-tile DMA. Deep x-tile buffering (bufs=8) keeps the DMA queue full so the
streaming phase is HBM-bound; ScalarE does exp + row-sum (accum_out) at
~1 elem/cycle/lane, below the DMA rate.
"""
import numpy as np
from contextlib import ExitStack

import concourse.bass as bass
import concourse.tile as tile
from concourse import bacc, mybir
from concourse.bass_utils import run_bass_kernel_spmd

B, V = 2048, 32000
N_CORES = 8
RPC = B // N_CORES            # 256 rows per core
P = 128                       # SBUF partitions
G = RPC // P                  # 2 row groups per core

# column tiling: small leading tiles (fast ScalarE start) for group 0,
# reversed for group 1 so the stream also ENDS on small tiles (short tail)
TILES = [500, 1500, 2000] + [4000] * 7
assert sum(TILES) == V
NT = len(TILES)
GTILES = [list(TILES), list(reversed(TILES))]

SCALE = 30.0
CM = SCALE * np.cos(0.5)      # 26.327476856711183
SM = SCALE * np.sin(0.5)      # 14.38276615812609
CB = SCALE * 0.35             # 10.5

F32 = mybir.dt.float32
AF = mybir.ActivationFunctionType
ALU = mybir.AluOpType

_cache = {}


def _build():
    nc = bacc.Bacc("TRN2", target_bir_lowering=False, debug=False,
                   num_devices=N_CORES)
    x = nc.dram_tensor("x", [RPC, V], F32, kind="ExternalInput")
    # out[:, 0:G] = den, out[:, G:2G] = num; the final ln(den)-num over the
    # 2048 per-row pairs happens host-side (saves the tail's ln-table load)
    out = nc.dram_tensor("out", [P, 2 * G], F32, kind="ExternalOutput")

    with tile.TileContext(nc) as tc, ExitStack() as ctx:
        xpool = ctx.enter_context(tc.tile_pool(name="x", bufs=8))
        epool = ctx.enter_context(tc.tile_pool(name="e", bufs=2))
        spool = ctx.enter_context(tc.tile_pool(name="s", bufs=1))

        ssum = spool.tile([P, G * NT], F32)   # per-(group,tile) exp row-sums
        tvec = spool.tile([P, G], F32)        # target logits t

        # --- target column + sqrt chain, emitted pre-stream: its ACT table
        # loads land in the ramp shadow while the first x tiles stream in ---
        with tc.high_priority():
            for g in range(G):
                nc.sync.dma_start(tvec[:, g:g + 1], x[g * P:(g + 1) * P, 0:1])

            tsq = spool.tile([P, G], F32)
            nc.vector.tensor_mul(tsq[:], tvec[:], tvec[:])
            omts = spool.tile([P, G], F32)
            # (t^2 * -1) + 1, clamped away from 0 for the sqrt
            nc.vector.tensor_scalar(omts[:], tsq[:], -1.0, 1.0,
                                    ALU.mult, ALU.add)
            omc = spool.tile([P, G], F32)
            nc.vector.tensor_scalar_max(omc[:], omts[:], 1e-30)
            r = spool.tile([P, G], F32)
            nc.scalar.activation(r[:], omc[:], AF.Sqrt)

        # --- streaming pass: exp(30 x) + per-row sums on ScalarE ---
        for g in range(G):
            rs = slice(g * P, (g + 1) * P)
            off = 0
            for t, tc_ in enumerate(GTILES[g]):
                xt = xpool.tile([P, tc_], F32, tag="xt")
                nc.sync.dma_start(xt[:], x[rs, off:off + tc_])
                et = epool.tile([P, tc_], F32, tag="et")
                nc.scalar.activation(
                    et[:], xt[:], AF.Exp, scale=SCALE,
                    accum_out=ssum[:, g * NT + t: g * NT + t + 1],
                )
                off += tc_

        # --- numerator epilogue (gap-fills into the stream; exp set stays) ---
        a = spool.tile([P, G], F32)
        nc.vector.tensor_scalar(a[:], tvec[:], float(CM), -float(CB),
                                ALU.mult, ALU.add)
        bb = spool.tile([P, G], F32)
        nc.vector.tensor_scalar_mul(bb[:], r[:], float(SM))
        num = spool.tile([P, G], F32)
        nc.vector.tensor_sub(num[:], a[:], bb[:])

        enum_ = spool.tile([P, G], F32)
        nc.scalar.activation(enum_[:], num[:], AF.Exp)
        e30t = spool.tile([P, G], F32)
        nc.scalar.activation(e30t[:], tvec[:], AF.Exp, scale=SCALE)
        # exp(num) - exp(30 t), folded before S arrives
        ed = spool.tile([P, G], F32)
        nc.vector.tensor_sub(ed[:], enum_[:], e30t[:])

        # --- tail: S, den, loss ---
        S = spool.tile([P, G], F32)
        for g in range(G):
            nc.vector.tensor_reduce(
                S[:, g:g + 1], ssum[:, g * NT:(g + 1) * NT],
                axis=mybir.AxisListType.X, op=ALU.add,
            )
        dn = spool.tile([P, 2 * G], F32)
        nc.vector.tensor_add(dn[:, 0:G], S[:], ed[:])
        nc.vector.tensor_copy(dn[:, G:2 * G], num[:])

        nc.sync.dma_start(out[:, :], dn[:])

    nc.compile()
    return nc


def _get_nc():
    if "nc" not in _cache:
        _cache["nc"] = _build()
    return _cache["nc"]


def _shard(preds, labels):
    """Rotate each row so its target column lands at column 0; split by core."""
    preds = np.ascontiguousarray(preds, dtype=np.float32)
    labels = np.asarray(labels).astype(np.int64)
    in_maps = []
    for c in range(N_CORES):
        shard = np.empty((RPC, V), np.float32)
        for i in range(RPC):
            r = c * RPC + i
            l = int(labels[r])
            shard[i, :V - l] = preds[r, l:]
            shard[i, V - l:] = preds[r, :l]
        in_maps.append({"x": shard})
    return in_maps


def kernel(preds, labels):
    in_maps = _shard(preds, labels)
    nc = _get_nc()
    res = run_bass_kernel_spmd(nc, in_maps, list(range(N_CORES)))
    total = 0.0
    for c in range(N_CORES):
        o = np.asarray(res.results[c]["out"], np.float64)
        den, num = o[:, :G], o[:, G:]
        total += (np.log(den) - num).sum()
    return np.array(total / B, dtype=np.float32)



# revision 2
# speedup vs baseline: 1.7769x; 1.7769x over previous
"""CosArcLoss on 8 TRN2 NeuronCores (Bass/Tile), fp8 two-region pipeline.

Math (reference, f32):
    t_i   = preds[i, labels[i]]
    num_i = 30*(cos(arccos(clip(t_i)) + 0.5) - 0.35)
    S_i   = sum_{j != labels[i]} exp(30*preds[i,j])
    loss  = mean_i( log(exp(num_i) + S_i) - num_i )

Device does all O(B*V) work: sum_j exp(30*q(x_ij)) over fp8-quantized
inputs (tolerance 2e-2 >> fp8 logsumexp bias ~2.4e-3). Host does the O(B)
epilogue: numerator from exact f32 targets, subtraction of the (exactly
simulated) target-column device contribution, final log/mean.

Per-core layout (256 rows, 32000 classes), split by class:
  region A (classes [0, CA)):  row-major fp8, HWDGE DMA, ScalarE exp
      (scale=30) with free accum_out row-sums. ~0.83 ns/elem on ACT.
  region B (classes [CA, V)):  transposed fp8 [class, row] chunks of 128
      classes, SWDGE DMA casts fp8->bf16 in flight, VectorE computes
      exp via the Schraudolph exp2 bit-trick (i16 = rint(x*S1+S2) whose
      bits ARE bf16(exp(30x)), 4x mode ~0.27 ns/elem), TensorE ones-matmul
      reduces over classes into 4 rotating PSUM banks.
All engines stream concurrently; DMA (HBM read 8.2MB + SDMA write 12.3MB
per core) is the roofline.
"""
import numpy as np
import ml_dtypes
from contextlib import ExitStack

import concourse.bass as bass
import concourse.tile as tile
from concourse import bacc, mybir
from concourse.bass_utils import run_bass_kernel_spmd

B, V = 2048, 32000
N_CORES = 8
RPC = B // N_CORES            # 256 rows per core
P = 128                       # SBUF partitions
G = RPC // P                  # 2 row groups (region A)

CA = 16000                    # classes handled by ACT (region A)
VB = V - CA                   # classes handled by DVE+TensorE (region B)
NCH = VB // P                 # 125 chunks of 128 classes
WA = 2000                     # region A tile width (classes)
NTA = CA // WA                # 8 tiles per group
KCH = 32                      # chunks per SWDGE cast DMA
DCH = 8                       # chunks per DVE schraudolph op
NBANK = 4                     # rotating PSUM accumulators

SCALE = 30.0
LN2 = float(np.log(2.0))
S1 = 128.0 * SCALE / LN2           # schraudolph slope (bf16 bits / x)
C0 = 0.0564005                     # zero-mean-rel-err offset
S2 = 128.0 * (127.0 - C0)

F32 = mybir.dt.float32
BF16 = mybir.dt.bfloat16
I16 = mybir.dt.int16
FP8 = mybir.dt.float8e4
AF = mybir.ActivationFunctionType
ALU = mybir.AluOpType
E4M3 = ml_dtypes.float8_e4m3

_cache = {}


def _build():
    nc = bacc.Bacc("TRN2", target_bir_lowering=False, debug=False,
                   num_devices=N_CORES)
    xa = nc.dram_tensor("xa", [RPC, CA], FP8, kind="ExternalInput")
    xbt = nc.dram_tensor("xbt", [P, NCH * RPC], FP8, kind="ExternalInput")
    # region A partial row-sums: [128, G*NTA]; region B bank sums: [1, 4*256]
    osa = nc.dram_tensor("osa", [P, G * NTA], F32, kind="ExternalOutput")
    osb = nc.dram_tensor("osb", [1, NBANK * RPC], F32, kind="ExternalOutput")

    with tile.TileContext(nc) as tc, ExitStack() as ctx:
        apool = ctx.enter_context(tc.tile_pool(name="ap", bufs=6))
        bpool = ctx.enter_context(tc.tile_pool(name="bp", bufs=2))
        epool = ctx.enter_context(tc.tile_pool(name="ep", bufs=2))
        ipool = ctx.enter_context(tc.tile_pool(name="ip", bufs=4))
        spool = ctx.enter_context(tc.tile_pool(name="sp", bufs=1))
        psum = ctx.enter_context(tc.tile_pool(name="ps", bufs=1, space="PSUM"))

        ssum = spool.tile([P, G * NTA], F32)
        ones = spool.tile([P, 1], BF16)
        nc.any.memset(ones[:], 1.0)
        banks = [psum.tile([P, RPC], F32, name=f"bank{k}") for k in range(NBANK)]

        # ---- region B: SWDGE cast DMAs; DVE schraudolph; TensorE reduce ----
        nmm = 0
        ch0 = 0
        while ch0 < NCH:
            k = min(KCH, NCH - ch0)
            xb = bpool.tile([P, KCH * RPC], BF16, tag="xb")
            nc.gpsimd.dma_start(xb[:, :k * RPC],
                                xbt[:, ch0 * RPC:(ch0 + k) * RPC])
            d0 = 0
            while d0 < k:
                d = min(DCH, k - d0)
                si = ipool.tile([P, DCH * RPC], I16, tag="si")
                nc.vector.tensor_scalar(si[:, :d * RPC],
                                        xb[:, d0 * RPC:(d0 + d) * RPC],
                                        S1, S2, ALU.mult, ALU.add)
                for j in range(d):
                    bk = banks[nmm % NBANK]
                    rhs = si[:, j * RPC:(j + 1) * RPC].bitcast(BF16)
                    nc.tensor.matmul(bk[:1], ones[:], rhs,
                                     start=(nmm < NBANK),
                                     stop=(nmm >= NCH - NBANK))
                    nmm += 1
                d0 += d
            ch0 += k

        # ---- region A: HWDGE fp8 tiles; ACT exp + accum_out row sums ----
        for t in range(NTA):
            for g in range(G):
                rs = slice(g * P, (g + 1) * P)
                xt = apool.tile([P, WA], FP8, tag="xt")
                nc.sync.dma_start(xt[:], xa[rs, t * WA:(t + 1) * WA])
                et = epool.tile([P, WA], BF16, tag="et")
                idx = g * NTA + t
                nc.scalar.activation(et[:], xt[:], AF.Exp, scale=SCALE,
                                     accum_out=ssum[:, idx:idx + 1])

        # ---- outputs ----
        sb = spool.tile([1, NBANK * RPC], F32)
        for kb in range(NBANK):
            nc.vector.tensor_copy(sb[:, kb * RPC:(kb + 1) * RPC], banks[kb][:1])
        nc.sync.dma_start(osb[:, :], sb[:])
        nc.sync.dma_start(osa[:, :], ssum[:])

    nc.compile()
    return nc


def _get_nc():
    if "nc" not in _cache:
        _cache["nc"] = _build()
    return _cache["nc"]


def _shard(preds, labels):
    """Quantize to fp8-e4m3 and build per-core region A/B device layouts."""
    preds = np.ascontiguousarray(preds, dtype=np.float32)
    q = preds.astype(E4M3)
    in_maps = []
    for c in range(N_CORES):
        rows = slice(c * RPC, (c + 1) * RPC)
        qa = np.ascontiguousarray(q[rows, :CA])
        # [256, VB] -> [VB, 256] -> chunks of 128 classes along free dim
        qb = np.ascontiguousarray(
            q[rows, CA:].T.reshape(NCH, P, RPC).transpose(1, 0, 2)
            .reshape(P, NCH * RPC))
        in_maps.append({"xa": qa, "xbt": qb})
    return in_maps


def kernel(preds, labels):
    preds = np.ascontiguousarray(preds, dtype=np.float32)
    labels = np.asarray(labels).astype(np.int64)
    in_maps = _shard(preds, labels)
    nc = _get_nc()
    res = run_bass_kernel_spmd(nc, in_maps, list(range(N_CORES)))

    # device row sums S (all classes, fp8-quantized)
    S = np.empty(B, dtype=np.float64)
    for c in range(N_CORES):
        r = res.results[c]
        sa = np.asarray(r["osa"], np.float64)          # [128, G*NTA]
        sb = np.asarray(r["osb"], np.float64)[0]       # [4*256]
        s_a = np.zeros(RPC)
        for g in range(G):
            s_a[g * P:(g + 1) * P] = sa[:, g * NTA:(g + 1) * NTA].sum(axis=1)
        s_b = sb.reshape(NBANK, RPC).sum(axis=0)
        S[c * RPC:(c + 1) * RPC] = s_a + s_b

    # subtract the device's own target-column contribution (exact simulation)
    idx = np.arange(B)
    tq32 = preds[idx, labels].astype(E4M3).astype(np.float32)
    in_a = labels < CA
    sub = np.empty(B, dtype=np.float64)
    sub[in_a] = np.exp(np.float64(SCALE) * tq32[in_a].astype(np.float64))
    vb = (tq32[~in_a] * np.float32(S1) + np.float32(S2)).astype(np.float32)
    i16 = np.rint(vb.astype(np.float64)).astype(np.int16)
    sub[~in_a] = i16.view(ml_dtypes.bfloat16).astype(np.float64)
    S_others = S - sub

    # numerator from exact f32 targets (reference formula)
    t = preds[idx, labels].astype(np.float64)
    eps = 1e-12
    theta = np.arccos(np.clip(t, -1.0 + eps, 1.0 - eps))
    theta = np.clip(theta, eps, np.pi - eps)
    num = SCALE * (np.cos(theta + 0.5) - 0.35)

    den = np.exp(num) + S_others
    loss = np.mean(np.log(den) - num)
    return np.array(loss, dtype=np.float32)


# revision 6
# speedup vs baseline: 1.8284x; 1.0290x over previous
"""CosArcLoss on 8 TRN2 NeuronCores (Bass/Tile), fp8 two-region pipeline.

Math (reference, f32):
    t_i   = preds[i, labels[i]]
    num_i = 30*(cos(arccos(clip(t_i)) + 0.5) - 0.35)
    S_i   = sum_{j != labels[i]} exp(30*preds[i,j])
    loss  = mean_i( log(exp(num_i) + S_i) - num_i )

Device does all O(B*V) work: sum_j exp(30*q(x_ij)) over fp8-quantized
inputs (tolerance 2e-2 >> fp8 logsumexp bias ~2.4e-3). Host does the O(B)
epilogue: numerator from exact f32 targets, subtraction of the (exactly
simulated) target-column device contribution, final log/mean.

Per-core layout (256 rows, 32000 classes), split by class:
  region A (classes [0, CA)):  row-major fp8, HWDGE DMA, ScalarE exp
      (scale=30) with free accum_out row-sums. ~0.83 ns/elem on ACT.
  region B (classes [CA, V)):  transposed fp8 [class, row] chunks of 128
      classes, SWDGE DMA casts fp8->bf16 in flight, VectorE computes
      exp via the Schraudolph exp2 bit-trick (i16 = rint(x*S1+S2) whose
      bits ARE bf16(exp(30x)), 4x mode ~0.27 ns/elem), TensorE ones-matmul
      reduces over classes into 4 rotating PSUM banks.
All engines stream concurrently; DMA (HBM read 8.2MB + SDMA write 12.3MB
per core) is the roofline.
"""
import numpy as np
import ml_dtypes
from contextlib import ExitStack

import concourse.bass as bass
import concourse.tile as tile
from concourse import bacc, mybir
from concourse.bass_utils import run_bass_kernel_spmd

B, V = 2048, 32000
N_CORES = 8
RPC = B // N_CORES            # 256 rows per core
P = 128                       # SBUF partitions
G = RPC // P                  # 2 row groups (region A)

CA = 14976                    # classes handled by ACT (region A)
VB = V - CA                   # classes handled by DVE+TensorE (region B)
NCH = VB // P                 # 133 chunks of 128 classes
WA = 1872                     # region A tile width (classes)
NTA = CA // WA                # 8 tiles per group
KCH = 16                      # chunks per SWDGE cast DMA
DCH = 8                       # chunks per DVE schraudolph op
NBANK = 4                     # rotating PSUM accumulators

SCALE = 30.0
LN2 = float(np.log(2.0))
S1 = 128.0 * SCALE / LN2           # schraudolph slope (bf16 bits / x)
C0 = 0.0564005                     # zero-mean-rel-err offset
S2 = 128.0 * (127.0 - C0)

F32 = mybir.dt.float32
BF16 = mybir.dt.bfloat16
I16 = mybir.dt.int16
FP8 = mybir.dt.float8e4
AF = mybir.ActivationFunctionType
ALU = mybir.AluOpType
E4M3 = ml_dtypes.float8_e4m3

_cache = {}


def _build():
    nc = bacc.Bacc("TRN2", target_bir_lowering=False, debug=False,
                   num_devices=N_CORES)
    xa = nc.dram_tensor("xa", [RPC, CA], FP8, kind="ExternalInput")
    xbt = nc.dram_tensor("xbt", [P, NCH * RPC], FP8, kind="ExternalInput")
    # region A partial row-sums: [128, G*NTA]; region B bank sums: [1, 4*256]
    osa = nc.dram_tensor("osa", [P, G * NTA], F32, kind="ExternalOutput")
    osb = nc.dram_tensor("osb", [1, NBANK * RPC], F32, kind="ExternalOutput")

    with tile.TileContext(nc) as tc, ExitStack() as ctx:
        apool = ctx.enter_context(tc.tile_pool(name="ap", bufs=12))
        bpool = ctx.enter_context(tc.tile_pool(name="bp", bufs=2))
        epool = ctx.enter_context(tc.tile_pool(name="ep", bufs=2))
        ipool = ctx.enter_context(tc.tile_pool(name="ip", bufs=4))
        spool = ctx.enter_context(tc.tile_pool(name="sp", bufs=1))
        psum = ctx.enter_context(tc.tile_pool(name="ps", bufs=1, space="PSUM"))

        ssum = spool.tile([P, G * NTA], F32)
        ones = spool.tile([P, 1], BF16)
        nc.any.memset(ones[:], 1.0)
        banks = [psum.tile([P, RPC], F32, name=f"bank{k}") for k in range(NBANK)]

        # ---- region A: HWDGE fp8 tiles; ACT exp + accum_out row sums ----
        # emitted first so the A-tile DMAs queue ahead of the SWDGE flood
        for t in range(NTA):
            for g in range(G):
                rs = slice(g * P, (g + 1) * P)
                xt = apool.tile([P, WA], FP8, tag="xt")
                nc.sync.dma_start(xt[:], xa[rs, t * WA:(t + 1) * WA])
                et = epool.tile([P, WA], BF16, tag="et")
                idx = g * NTA + t
                nc.scalar.activation(et[:], xt[:], AF.Exp, scale=SCALE,
                                     accum_out=ssum[:, idx:idx + 1])

        # ---- region B: SWDGE cast DMAs; DVE schraudolph; TensorE reduce ----
        nmm = 0
        ch0 = 0
        while ch0 < NCH:
            k = min(KCH, NCH - ch0)
            xb = bpool.tile([P, KCH * RPC], BF16, tag="xb")
            nc.gpsimd.dma_start(xb[:, :k * RPC],
                                xbt[:, ch0 * RPC:(ch0 + k) * RPC])
            d0 = 0
            while d0 < k:
                d = min(DCH, k - d0)
                si = ipool.tile([P, DCH * RPC], I16, tag="si")
                nc.vector.tensor_scalar(si[:, :d * RPC],
                                        xb[:, d0 * RPC:(d0 + d) * RPC],
                                        S1, S2, ALU.mult, ALU.add)
                for j in range(d):
                    bk = banks[nmm % NBANK]
                    rhs = si[:, j * RPC:(j + 1) * RPC].bitcast(BF16)
                    nc.tensor.matmul(bk[:1], ones[:], rhs,
                                     start=(nmm < NBANK),
                                     stop=(nmm >= NCH - NBANK))
                    nmm += 1
                d0 += d
            ch0 += k

        # ---- outputs ----
        sb = spool.tile([1, NBANK * RPC], F32)
        for kb in range(NBANK):
            nc.vector.tensor_copy(sb[:, kb * RPC:(kb + 1) * RPC], banks[kb][:1])
        nc.sync.dma_start(osb[:, :], sb[:])
        nc.sync.dma_start(osa[:, :], ssum[:])

    nc.compile()
    return nc


def _get_nc():
    if "nc" not in _cache:
        _cache["nc"] = _build()
    return _cache["nc"]


def _shard(preds, labels):
    """Quantize to fp8-e4m3 and build per-core region A/B device layouts."""
    preds = np.ascontiguousarray(preds, dtype=np.float32)
    q = preds.astype(E4M3)
    in_maps = []
    for c in range(N_CORES):
        rows = slice(c * RPC, (c + 1) * RPC)
        qa = np.ascontiguousarray(q[rows, :CA])
        # [256, VB] -> [VB, 256] -> chunks of 128 classes along free dim
        qb = np.ascontiguousarray(
            q[rows, CA:].T.reshape(NCH, P, RPC).transpose(1, 0, 2)
            .reshape(P, NCH * RPC))
        in_maps.append({"xa": qa, "xbt": qb})
    return in_maps


def kernel(preds, labels):
    preds = np.ascontiguousarray(preds, dtype=np.float32)
    labels = np.asarray(labels).astype(np.int64)
    in_maps = _shard(preds, labels)
    nc = _get_nc()
    res = run_bass_kernel_spmd(nc, in_maps, list(range(N_CORES)))

    # device row sums S (all classes, fp8-quantized)
    S = np.empty(B, dtype=np.float64)
    for c in range(N_CORES):
        r = res.results[c]
        sa = np.asarray(r["osa"], np.float64)          # [128, G*NTA]
        sb = np.asarray(r["osb"], np.float64)[0]       # [4*256]
        s_a = np.zeros(RPC)
        for g in range(G):
            s_a[g * P:(g + 1) * P] = sa[:, g * NTA:(g + 1) * NTA].sum(axis=1)
        s_b = sb.reshape(NBANK, RPC).sum(axis=0)
        S[c * RPC:(c + 1) * RPC] = s_a + s_b

    # subtract the device's own target-column contribution (exact simulation)
    idx = np.arange(B)
    tq32 = preds[idx, labels].astype(E4M3).astype(np.float32)
    in_a = labels < CA
    sub = np.empty(B, dtype=np.float64)
    sub[in_a] = np.exp(np.float64(SCALE) * tq32[in_a].astype(np.float64))
    vb = (tq32[~in_a] * np.float32(S1) + np.float32(S2)).astype(np.float32)
    i16 = np.rint(vb.astype(np.float64)).astype(np.int16)
    sub[~in_a] = i16.view(ml_dtypes.bfloat16).astype(np.float64)
    S_others = S - sub

    # numerator from exact f32 targets (reference formula)
    t = preds[idx, labels].astype(np.float64)
    eps = 1e-12
    theta = np.arccos(np.clip(t, -1.0 + eps, 1.0 - eps))
    theta = np.clip(theta, eps, np.pi - eps)
    num = SCALE * (np.cos(theta + 0.5) - 0.35)

    den = np.exp(num) + S_others
    loss = np.mean(np.log(den) - num)
    return np.array(loss, dtype=np.float32)
